# revision 3
# baseline (speedup 1.0000x reference)
"""Causal self-attention on 8 Trainium2 NeuronCores.

Sharding: tensor-parallel over heads (16 heads -> 2 heads per core).
Each core computes q/k/v projections for its 2 heads, causal attention,
and a partial out-projection (rows of w_out for its heads). The host
sums the 8 partial [4096, 1024] outputs (the TP all-reduce).

v2: fully fused single-pipeline schedule. The projection matmuls for
batch 1 and the deferred out-projection matmuls are emitted at lower
priority than the attention chain, so the Tile scheduler uses them as
TensorE filler during the exp(ScalarE)-bound attention windows. This
keeps PE dense (no HAM re-throttle) and hides the softmax serial chain.

Per-core dataflow (all matmuls bf16, fp32 PSUM):
  proj:  qT/kT/vT = w^T @ x^T in [128=2h*64, 4096] layout, K=128 x 8
         accumulation chunks, N=512 moving tiles; vT is DMA-transposed
         into v_aug = [v_h0|1 , v_h1|1] per 128-row chunk (the ones
         column makes the PV matmul also emit softmax denominators).
  attn per (batch, q-chunk of 512):
         S^T pair: the two heads' k@q^T matmuls occupy partition rows
         0-63 / 64-127 -> the PE runs them concurrently (row groups);
         exp in ONE activation per k-chunk (scores ~N(0,1), no
         max-shift needed); causal zeroing via gpsimd affine_select on
         only the 128-wide diagonal strip; fully-masked leading columns
         of diagonal tiles skipped everywhere; O^T/denoms = [v|1]^T@P^T
         accumulated over k-chunks.
  norm:  ps_O evacuated once per head ([65,512] incl. denom row),
         reciprocal taken directly on the denominator row, broadcast
         over partitions via a small DRAM bounce, one DVE multiply
         produces normalized aT.
  out:   y rows stream through w_out per 128-row chunk; 1 batched
         y-store DMA per q-chunk.
"""

import numpy as np
import ml_dtypes

import concourse.bacc as bacc
import concourse.mybir as mybir
from concourse.tile import TileContext
from concourse.bass_utils import run_bass_kernel_spmd

BF16 = mybir.dt.bfloat16
F32 = mybir.dt.float32
AF = mybir.ActivationFunctionType
ALU = mybir.AluOpType

NP_BF16 = np.dtype(ml_dtypes.bfloat16)

B, T, D_MODEL = 2, 2048, 1024
N_HEADS, HEAD_DIM = 16, 64
N_CORES = 8
HPC = N_HEADS // N_CORES          # heads per core (2)
DH = HEAD_DIM
HD = HPC * DH                     # 128 head-dims per core
SCALE = 1.0 / float(np.sqrt(DH))  # 0.125

QC = 512                          # q-chunk (free dim of S^T tiles)
KC = 128                          # k-chunk (partition dim of S^T tiles)


def build_program(b=B, t=T, d=D_MODEL):
    rows = b * t
    dch = d // 128                # contraction chunks for the projections
    ng_w = 1024                   # x^T column-group width per proj pass
    ngrp = rows // ng_w           # 4
    rcpg = ng_w // 128            # row-chunks per group (8)
    nqc = t // QC                 # q-chunks per batch (4)
    rpq = QC // KC                # k-chunks per q-chunk (4)
    n_rchunk = rows // 128        # 32
    assert t % QC == 0 and d % 512 == 0 and rows % ng_w == 0

    nc = bacc.Bacc("TRN2", target_bir_lowering=False, debug=False,
                   num_devices=N_CORES)

    xT_d = nc.dram_tensor("xT", [d, rows], BF16, kind="ExternalInput")
    wqkv_d = nc.dram_tensor("wqkv", [d, 3 * HD], BF16, kind="ExternalInput")
    wo_d = nc.dram_tensor("wo", [HD, d], BF16, kind="ExternalInput")
    y_d = nc.dram_tensor("y", [rows, d], BF16, kind="ExternalOutput")

    # batched-DMA views
    xv = xT_d.rearrange("(i k p) m -> p i k m", k=2, p=128)   # [128,4,2,rows]
    yv = y_d.rearrange("(c r p) d -> p c r d", r=4, p=128)    # [128,8,4,d]

    with TileContext(nc) as tc:
        with tc.tile_pool(name="persist", bufs=1) as pp, \
             tc.tile_pool(name="xt", bufs=16) as pxt, \
             tc.tile_pool(name="pt", bufs=8) as ppt, \
             tc.tile_pool(name="nrm", bufs=4) as pn, \
             tc.tile_pool(name="dramtmp", bufs=4, space="DRAM") as pd, \
             tc.tile_pool(name="ysb", bufs=2) as py, \
             tc.tile_pool(name="psum", bufs=2, space="PSUM") as pps:
            wqkv = pp.tile([128, dch, 3 * HD], BF16)
            wo = pp.tile([HD, d], BF16)
            qT = pp.tile([HD, rows], BF16)
            kT = pp.tile([HD, rows], BF16)
            vT = pp.tile([HD, rows], BF16)
            v_nat = pp.tile([128, n_rchunk, HD], BF16)
            v_aug = pp.tile([128, n_rchunk, HPC, DH + 1], BF16)
            aTn = pp.tile([HD, rows], BF16)

            nc.sync.dma_start(wqkv[:], wqkv_d.rearrange("(k p) m -> p k m", p=128))
            nc.sync.dma_start(wo[:], wo_d[:])
            nc.any.memset(v_aug[:], 1.0)

            xts = {}

            def emit_load(g):
                c0 = g * ng_w
                tiles = []
                for i in range(4):
                    xt = pxt.tile([128, 2, ng_w], BF16, tag="xt", name="xt")
                    nc.sync.dma_start(xt[:], xv[:, i, :, c0:c0 + ng_w])
                    tiles.append(xt)
                xts[g] = tiles

            def emit_proj(g):
                c0 = g * ng_w
                for m in (2, 0, 1):               # v first (feeds transpose)
                    dst = (qT, kT, vT)[m]
                    for n2 in range(ng_w // 512):
                        ps = pps.tile([128, 512], F32, tag="mm", name="mm")
                        for kc in range(dch):
                            nc.tensor.matmul(
                                ps[:],
                                wqkv[:, kc, m * 128:(m + 1) * 128],
                                xts[g][kc // 2][:, kc % 2,
                                                n2 * 512:(n2 + 1) * 512],
                                start=(kc == 0), stop=(kc == dch - 1))
                        nc.vector.tensor_copy(
                            dst[:, c0 + n2 * 512:c0 + (n2 + 1) * 512], ps[:])
                    if m == 2:
                        r0 = g * rcpg
                        nc.sync.dma_start_transpose(
                            v_nat[:, r0:r0 + rcpg, :], vT[:, c0:c0 + ng_w])
                        for h in range(HPC):
                            nc.vector.tensor_copy(
                                v_aug[:, r0:r0 + rcpg, h, 0:DH],
                                v_nat[:, r0:r0 + rcpg, h * DH:(h + 1) * DH])

            def emit_attn(bi, qc):
                """S/exp/mask/PV chain + normalization for one q-chunk.

                Returns the out-projection emitter so it can be deferred
                (emitted later = lower scheduler priority = PE filler for
                the later attention windows)."""
                q0 = bi * t + qc * QC
                ps_O = [pps.tile([DH + 1, QC], F32, tag=f"sO{h}", bufs=1,
                                 name=f"sO{h}")
                        for h in range(HPC)]
                kpq = rpq * (qc + 1)
                for kc in range(kpq):
                    k0 = bi * t + kc * KC
                    grc = k0 // 128
                    v0 = max(0, (kc - rpq * qc) * KC)
                    ps_S = pps.tile([128, HPC, QC], F32, tag="sS", name="sS")
                    for h in range(HPC):
                        # heads at partition rows 0-63 / 64-127: the PE
                        # runs this pair concurrently (disjoint row grps)
                        nc.tensor.matmul(
                            ps_S[:, h, v0:],
                            kT[h * DH:(h + 1) * DH, k0:k0 + KC],
                            qT[h * DH:(h + 1) * DH, q0 + v0:q0 + QC],
                            start=True, stop=True)
                    pt = ppt.tile([128, HPC, QC], BF16, tag="pt", name="pt")
                    nc.scalar.activation(pt[:, :, v0:], ps_S[:, :, v0:],
                                         AF.Exp, scale=SCALE)
                    if kc >= rpq * qc:
                        # only the 128-wide diagonal strip is ambiguous;
                        # base telescopes to 0 there (q0+v0 == k0)
                        nc.gpsimd.affine_select(
                            out=pt[:, :, v0:v0 + KC], in_=pt[:, :, v0:v0 + KC],
                            compare_op=ALU.is_ge, fill=0.0,
                            base=0, pattern=[[0, HPC], [1, KC]],
                            channel_multiplier=-1)
                    for h in range(HPC):
                        nc.tensor.matmul(
                            ps_O[h][:, v0:],
                            v_aug[:, grc, h, :],
                            pt[:, h, v0:],
                            start=(kc == 0), stop=(kc == kpq - 1))

                # ---- normalize this q-chunk ----
                usts = []
                rsbs = []
                for h in range(HPC):
                    ust = pn.tile([DH + 1, QC], BF16, tag=f"ust{h}",
                                  name=f"ust{h}")
                    nc.vector.tensor_copy(ust[:], ps_O[h][:])
                    usts.append(ust)
                    rsbs.append(pn.tile([1, QC], BF16, tag=f"rsb{h}",
                                        name=f"rsb{h}"))
                with nc.allow_low_precision(
                        reason="softmax denominators are O(100) and the "
                               "output is bf16 anyway"):
                    for h in range(HPC):
                        nc.vector.reciprocal(rsbs[h][:],
                                             usts[h][DH:DH + 1, :])
                r_d = pd.tile([HPC, QC], BF16, name="r_d")
                for h in range(HPC):
                    nc.sync.dma_start(r_d[h:h + 1, :], rsbs[h][:])
                for h in range(HPC):
                    rbc = pn.tile([DH, QC], BF16, tag=f"rbc{h}",
                                  name=f"rbc{h}")
                    nc.sync.dma_start(rbc[:],
                                      r_d[h:h + 1, :].to_broadcast((DH, QC)))
                    nc.vector.tensor_mul(
                        aTn[h * DH:(h + 1) * DH, q0:q0 + QC],
                        usts[h][0:DH, :], rbc[:])

                def emit_outproj():
                    ysb = py.tile([128, rpq, d], BF16, tag="ysb", name="ysb")
                    for rc4 in range(rpq):
                        for n2 in range(d // 512):
                            ps_y = pps.tile([128, 512], F32, tag="mm",
                                            name="mm")
                            nc.tensor.matmul(
                                ps_y[:],
                                aTn[:, q0 + rc4 * 128:q0 + (rc4 + 1) * 128],
                                wo[:, n2 * 512:(n2 + 1) * 512],
                                start=True, stop=True)
                            nc.vector.tensor_copy(
                                ysb[:, rc4, n2 * 512:(n2 + 1) * 512], ps_y[:])
                    nc.sync.dma_start(yv[:, q0 // QC], ysb[:])
                return emit_outproj

            # ---- emission schedule (priority order for the scheduler) ----
            for g in range(ngrp):
                emit_load(g)
            emit_proj(0)
            emit_proj(1)
            op03 = emit_attn(0, 3)
            emit_proj(2)          # PE filler inside batch-0 attention
            emit_proj(3)
            op02 = emit_attn(0, 2)
            op01 = emit_attn(0, 1)
            op00 = emit_attn(0, 0)
            op13 = emit_attn(1, 3)
            op03()                # deferred out-projections fill batch-1
            op02()
            op12 = emit_attn(1, 2)
            op01()
            op00()
            op11 = emit_attn(1, 1)
            op13()
            op10 = emit_attn(1, 0)
            op12()
            op11()
            op10()

    nc.compile()
    return nc


def make_in_maps(x, w_qkv, w_out, b=B, t=T, d=D_MODEL):
    rows = b * t
    xr = np.asarray(x, dtype=np.float32).reshape(rows, d)
    xT = np.ascontiguousarray(xr.T).astype(NP_BF16)
    wq = np.asarray(w_qkv[:, 0:d]).reshape(d, N_HEADS, DH)
    wk = np.asarray(w_qkv[:, d:2 * d]).reshape(d, N_HEADS, DH)
    wvf = np.asarray(w_qkv[:, 2 * d:3 * d]).reshape(d, N_HEADS, DH)
    in_maps = []
    for c in range(N_CORES):
        h0, h1 = HPC * c, HPC * c + HPC
        wqkv_c = np.concatenate(
            [wq[:, h0:h1].reshape(d, HD), wk[:, h0:h1].reshape(d, HD),
             wvf[:, h0:h1].reshape(d, HD)], axis=1).astype(NP_BF16)
        wo_c = np.ascontiguousarray(w_out[h0 * DH:h1 * DH, :]).astype(NP_BF16)
        in_maps.append({"xT": xT, "wqkv": wqkv_c, "wo": wo_c})
    return in_maps


_PROGRAM_CACHE = {}


def _get_program():
    if "nc" not in _PROGRAM_CACHE:
        _PROGRAM_CACHE["nc"] = build_program()
    return _PROGRAM_CACHE["nc"]


def run(x, w_qkv, w_out, trace=False, tmpdir=None):
    nc = _get_program()
    in_maps = make_in_maps(x, w_qkv, w_out)
    res = run_bass_kernel_spmd(nc, in_maps, list(range(N_CORES)), trace=trace,
                               tmpdir=tmpdir)
    parts = np.stack([np.asarray(res.results[c]["y"], dtype=np.float32)
                      for c in range(N_CORES)])
    y = parts.sum(axis=0).reshape(B, T, D_MODEL)
    return y, res


def kernel(x, w_qkv, w_out):
    y, _ = run(x, w_qkv, w_out)
    return y
